# revision 9
# baseline (speedup 1.0000x reference)
"""Trainium2 Bass kernel for nn_DecoderRNN (LSTM decoder + big vocab projection).

Strategy (8 NeuronCores, SPMD):
  - The LSTM recurrence (B=32, T=64, H=512) is replicated on every core:
    its per-step cost is dominated by streaming W_hh through the PE, which is
    batch-size independent, so sharding batch would not help, and sharding the
    gate dim across cores loses to per-step h-exchange latency.
  - The output projection (fc) is tensor-parallel over the vocab dim:
    core c computes logits[:, :, 1250*c : 1250*(c+1)] and writes its own
    [32, 64, 1250] output slab; the host concatenates slabs.
  - Embedding lookup runs on-device via indirect (gather) DMA, one tile per
    step so it never blocks the gpsimd engine's per-step work.
  - The input projection Xp = xs @ W_ih.T + b is computed in bulk (it has no
    recurrent dependency) into an SBUF ring, then PRELOADED into the gate
    PSUM banks by gpsimd one step ahead. The recurrent matmuls accumulate on
    top with start=False, so no PE cycles are spent injecting Xp.

Per-step pipeline (steady state ~5.5-6us):
  - 4 persistent PSUM banks hold the gates [32, 512] per chunk, column order
    [g | i | f | o] (host-permuted) so tanh(g), sigma(i), sigma(f) complete
    while the o matmuls still stream; o is computed in two 256-wide halves.
  - c = sigma(f)*c + sigma(i)*tanh(g) runs in two half-lanes: half 0 on the
    vector engine, half 1 on gpsimd, overlapped with the o matmuls.
  - After sigma(o): h = sigma(o)*tanh(c) per half, PE-transposed into hsT.
  - fc chunks and Xp production matmuls fill the PE while the elementwise
    tail of the step drains.

kernel(**inputs) takes FULL unsharded inputs, returns FULL [32, 64, 10000].
"""

import sys

sys.path.insert(0, "/opt/trn_rl_repo")

import numpy as np

N_CORES = 8
B, T = 32, 64
E, H, V = 512, 512, 10000
G4 = 4 * H            # 2048
TB = T * B            # 2048
VSL = V // N_CORES    # 1250 vocab rows per core
VPAD = 1280           # padded so fc N-chunks are 512/512/256 (all >=256)

_PROGRAM = None


def _build_program():
    import concourse.bass as bass
    import concourse.tile as tile
    from concourse import bacc, mybir
    from concourse.masks import make_identity
    from contextlib import ExitStack

    f32 = mybir.dt.float32
    bf16 = mybir.dt.bfloat16
    i32 = mybir.dt.int32
    AF = mybir.ActivationFunctionType

    nc = bacc.Bacc(
        "TRN2",
        target_bir_lowering=False,
        debug=False,
        num_devices=N_CORES,
    )

    features = nc.dram_tensor("features", [B, E], f32, kind="ExternalInput").ap()
    idx = nc.dram_tensor("idx", [TB], i32, kind="ExternalInput").ap()
    embed = nc.dram_tensor("embed", [V, E], f32, kind="ExternalInput").ap()
    wihT = nc.dram_tensor("wihT", [E, G4], bf16, kind="ExternalInput").ap()
    whhT = nc.dram_tensor("whhT", [H, G4], bf16, kind="ExternalInput").ap()
    bih = nc.dram_tensor("bih", [G4], bf16, kind="ExternalInput").ap()
    bhh = nc.dram_tensor("bhh", [G4], bf16, kind="ExternalInput").ap()
    fcwT = nc.dram_tensor("fcwT", [H, VPAD], bf16, kind="ExternalInput").ap()
    fcb = nc.dram_tensor("fcb", [VPAD], bf16, kind="ExternalInput").ap()
    onesv = nc.dram_tensor("onesv", [128], bf16, kind="ExternalInput").ap()
    out = nc.dram_tensor("out", [B, T, VSL], f32, kind="ExternalOutput").ap()
    # Output viewed as [t, b, v]: a 128-row t-major tb tile = 4 t planes.
    out_r = out.rearrange("b t v -> t b v")

    # gate column order (host permutes rows of W/bias to match):
    # [0:512) = g, [512:1024) = i, [1024:1536) = f, [1536:2048) = o
    CG, CI, CF, CO = 0, 1, 2, 3

    def col(c):
        return slice(512 * c, 512 * (c + 1))

    with tile.TileContext(nc) as tc, ExitStack() as ctx:
        # ---------------- persistent state ----------------
        state = ctx.enter_context(tc.tile_pool(name="state", bufs=1))
        # h.T history: block t holds h(t).T (written at the end of step t).
        # Layout [p, k, 32*t + b] = h(t)[b, 128*k + p]
        hsT = state.tile([128, 4, 32 * T], bf16, tag="hsT")
        whhT_sb = state.tile([128, 4, G4], bf16, tag="whhT")
        xsT = state.tile([128, 4, TB], bf16, tag="xsT")   # [p, k, tb] = xs[tb, 128k+p]
        wihT_sb = state.tile([128, 4, G4], bf16, tag="wihT")
        fcwT_sb = state.tile([128, 4, VPAD], bf16, tag="fcwT")
        fcb_sb = state.tile([1, VPAD], bf16, tag="fcb")
        bias128 = state.tile([128, G4], bf16, tag="bias128")
        fcb128 = state.tile([128, VPAD], f32, tag="fcb128")
        c_sb = state.tile([B, H], f32, tag="c")
        ident_b = state.tile([128, 128], bf16, tag="ident_b")
        ones = state.tile([1, 128], bf16, tag="ones")
        xs_b = state.tile([128, 16, E], bf16, tag="xs_b")
        idx_sb = state.tile([128, 16], i32, tag="idx")

        # persistent PSUM: 4 gate banks, never start=True (preload overwrites)
        gstate = ctx.enter_context(tc.tile_pool(name="gstate", bufs=1, space="PSUM"))
        gb = []
        for c in range(4):
            gbank_c = gstate.tile([B, 512], f32, tag=f"g{c}", name=f"gbank{c}")
            gb.append(gbank_c)

        fc_psum = ctx.enter_context(tc.tile_pool(name="fc_ps", bufs=1, space="PSUM"))
        xp_psum = ctx.enter_context(tc.tile_pool(name="xp_ps", bufs=1, space="PSUM"))
        tp_psum = ctx.enter_context(tc.tile_pool(name="tp_ps", bufs=2, space="PSUM"))

        make_identity(nc, ident_b[:])
        nc.vector.memset(c_sb[:], 0.0)

        nc.sync.dma_start(fcb_sb[:], fcb[None, :])
        nc.sync.dma_start(ones[:], onesv[None, :])

        # ---------------- Xp production machinery ----------------
        # Produced m-tiles [128, 2048] live in an SBUF ring; the gpsimd
        # preload of step t copies rows [32*(t%4), +32) of tile t//4 into the
        # gate PSUM banks.
        xp_ring = ctx.enter_context(tc.tile_pool(name="xp_ring", bufs=2))
        xp_tiles = {}

        def xp_chunk_mms(m, cch, ks=(0, 1, 2, 3)):
            """PE part of one Xp chunk: psum += xs_tile @ W_ih.T[:, chunk]."""
            sl = col(cch)
            if ks[0] == 0:
                ps = xp_psum.tile([128, 512], f32, tag="xp")
                xp_chunk_mms.cur = ps
            else:
                ps = xp_chunk_mms.cur
            for k in ks:
                nc.tensor.matmul(
                    ps[:],
                    lhsT=xsT[:, k, 128 * m : 128 * (m + 1)],
                    rhs=wihT_sb[:, k, sl],
                    start=(k == 0),
                    stop=(k == 3),
                )
            return ps

        def xp_chunk_add(m, cch, ps):
            sl = col(cch)
            nc.vector.tensor_add(xp_tiles[m][:, sl], ps[:], bias128[:, sl])

        def produce_xp(m):
            xp_m_tile = xp_ring.tile([128, G4], bf16, tag="xp_m")
            xp_tiles[m] = xp_m_tile
            for cch in range(4):
                ps = xp_chunk_mms(m, cch)
                xp_chunk_add(m, cch, ps)

        # Preloads write PSUM, so they must run on scalar (Activation) or
        # vector (DVE) — GPSIMD cannot access PSUM on TRN2.
        def preload_scalar(t, c):
            m, q = t // 4, t % 4
            nc.scalar.copy(gb[c][:, :], xp_tiles[m][32 * q : 32 * (q + 1), col(c)])

        def preload_vector(t, c):
            m, q = t // 4, t % 4
            nc.vector.tensor_copy(gb[c][:, :], xp_tiles[m][32 * q : 32 * (q + 1), col(c)])

        def preload(t):
            preload_scalar(t, 0)
            for c in (1, 2, 3):
                preload_vector(t, c)

        # ---------------- embedding gather + xs.T transposes ----------------
        def gather(m):
            nc.gpsimd.indirect_dma_start(
                out=xs_b[:, m, :],
                out_offset=None,
                in_=embed[:, :],
                in_offset=bass.IndirectOffsetOnAxis(ap=idx_sb[:, m : m + 1], axis=0),
            )

        def transpose_m(m):
            for e in range(4):
                pt = tp_psum.tile([128, 128], bf16, tag="pt")
                nc.tensor.transpose(
                    pt[:], xs_b[:, m, 128 * e : 128 * (e + 1)], ident_b[:]
                )
                nc.vector.tensor_copy(xsT[:, e, 128 * m : 128 * (m + 1)], pt[:])

        # ---------------- prologue ----------------
        with ExitStack() as pro:
            small_pool = pro.enter_context(tc.tile_pool(name="small", bufs=1))
            bias1_sb = small_pool.tile([1, G4], bf16, tag="bias1")
            bias2_sb = small_pool.tile([1, G4], bf16, tag="bias2")

            nc.sync.dma_start(idx_sb[:], idx.rearrange("(m p) -> p m", p=128))
            nc.sync.dma_start(bias1_sb[:], bih[None, :])
            nc.sync.dma_start(bias2_sb[:], bhh[None, :])
            # wihT first (needed by the first Xp tiles), then whhT (step 1),
            # fcwT last (first used around step 4)
            nc.sync.dma_start(wihT_sb[:], wihT.rearrange("(k p) g -> p k g", p=128))
            nc.sync.dma_start(whhT_sb[:], whhT.rearrange("(k p) g -> p k g", p=128))
            nc.sync.dma_start(fcwT_sb[:], fcwT.rearrange("(k p) v -> p k v", p=128))

            # bias128 = broadcast(b_ih + b_hh) via rank-1 matmuls
            for cch in range(4):
                sl = col(cch)
                bp = xp_psum.tile([128, 512], f32, tag="xp")
                nc.tensor.matmul(bp[:], lhsT=ones[0:1, :], rhs=bias1_sb[0:1, sl],
                                 start=True, stop=False)
                nc.tensor.matmul(bp[:], lhsT=ones[0:1, :], rhs=bias2_sb[0:1, sl],
                                 start=False, stop=True)
                nc.vector.tensor_copy(bias128[:, sl], bp[:])
            # fcb128 = broadcast(fc_b) via rank-1 matmuls
            for c0, csz in ((0, 512), (512, 512), (1024, 256)):
                bp = xp_psum.tile([128, 512], f32, tag="xp")
                nc.tensor.matmul(bp[:, 0:csz], lhsT=ones[0:1, :],
                                 rhs=fcb_sb[0:1, c0 : c0 + csz], start=True, stop=True)
                nc.vector.tensor_copy(fcb128[:, c0 : c0 + csz], bp[:, 0:csz])

            # first xs tiles: gather 0 (covers steps 1..4 inputs), features
            # overwrite rows 0..31 (step 0 input)
            gather(0)
            nc.gpsimd.dma_start(xs_b[0:32, 0, :], features[:, :])
            gather(1)
            transpose_m(0)
            produce_xp(0)
            preload(0)
            transpose_m(1)

        # ---------------- main recurrence + interleaved fc/Xp ----------------
        work = ctx.enter_context(tc.tile_pool(name="work", bufs=2))
        lg_pool = ctx.enter_context(tc.tile_pool(name="lg", bufs=2))

        FC_CHUNKS = ((0, 512), (512, 512), (1024, 256))
        lg_tiles = {}

        def fc_chunk_mms(m, j):
            if j == 0:
                lg_new = lg_pool.tile([128, VPAD], f32, tag="lg")
                lg_tiles[m] = lg_new
            c0, csz = FC_CHUNKS[j]
            fps = fc_psum.tile([128, 512], f32, tag="fc")
            for k in range(4):
                nc.tensor.matmul(
                    fps[:, 0:csz],
                    lhsT=hsT[:, k, 128 * m : 128 * (m + 1)],
                    rhs=fcwT_sb[:, k, c0 : c0 + csz],
                    start=(k == 0),
                    stop=(k == 3),
                )
            return fps

        def fc_chunk_finish(m, j, fps):
            c0, csz = FC_CHUNKS[j]
            nc.vector.tensor_add(
                lg_tiles[m][:, c0 : c0 + csz], fps[:, 0:csz], fcb128[:, c0 : c0 + csz]
            )
            if j == 2:
                nc.sync.dma_start(
                    out_r[4 * m : 4 * (m + 1), :, :], lg_tiles[m][:, 0:VSL]
                )

        for t in range(T):
            q = t % 4
            xp_m_next = t // 4 + 1

            nl = work.tile([B, G4], f32, tag="nl")
            tanhc = work.tile([B, H], f32, tag="tanhc")
            fmul = work.tile([B, H], f32, tag="fmul")
            ig = work.tile([B, H], f32, tag="ig")
            h_t = work.tile([B, H], bf16, tag="h")

            def gate_mms(cch, c0=0, csz=512):
                if t == 0:
                    return  # gates(0) = Xp(0) already preloaded, no h yet
                for k in range(4):
                    nc.tensor.matmul(
                        gb[cch][:, c0 : c0 + csz],
                        lhsT=hsT[:, k, 32 * (t - 1) : 32 * t],
                        rhs=whhT_sb[:, k, 512 * cch + c0 : 512 * cch + c0 + csz],
                        start=False,
                        stop=(k == 3),
                        skip_group_check=True,
                    )

            # --- PE: g, i, f chunk matmuls; scalar acts interleaved ---
            gate_mms(CG)
            # lazy gather early in the gpsimd queue (engine idle here):
            # tile t+2 gathered at step t, transposed at step t+1
            if t + 2 <= 15:
                gather(t + 2)
            nc.scalar.activation(nl[:, col(CG)], gb[CG][:], AF.Tanh)
            gate_mms(CI)
            nc.scalar.activation(nl[:, col(CI)], gb[CI][:], AF.Sigmoid)
            # vector lane half0 + gpsimd lane half1: ig = tanh(g)*sigma(i)
            nc.vector.tensor_mul(ig[:, 0:256], nl[:, 0:256], nl[:, 512:768])
            nc.gpsimd.tensor_mul(ig[:, 256:512], nl[:, 256:512], nl[:, 768:1024])
            gate_mms(CF)
            nc.scalar.activation(nl[:, col(CF)], gb[CF][:], AF.Sigmoid)
            nc.vector.tensor_mul(fmul[:, 0:256], nl[:, 1024:1280], c_sb[:, 0:256])
            nc.gpsimd.tensor_mul(fmul[:, 256:512], nl[:, 1280:1536], c_sb[:, 256:512])
            # --- o in two halves ---
            gate_mms(CO, 0, 256)
            nc.scalar.activation(nl[:, 1536:1792], gb[CO][:, 0:256], AF.Sigmoid)
            gate_mms(CO, 256, 256)
            nc.scalar.activation(nl[:, 1792:2048], gb[CO][:, 256:512], AF.Sigmoid)

            # c = fmul + ig (half lanes), tanh(c) on scalar
            nc.vector.tensor_add(c_sb[:, 0:256], fmul[:, 0:256], ig[:, 0:256])
            nc.scalar.activation(tanhc[:, 0:256], c_sb[:, 0:256], AF.Tanh)
            nc.gpsimd.tensor_add(c_sb[:, 256:512], fmul[:, 256:512], ig[:, 256:512])
            nc.scalar.activation(tanhc[:, 256:512], c_sb[:, 256:512], AF.Tanh)

            # --- PE fillers: fc chunk + first half of xp production ---
            m_fc = t // 4 - 1
            fc_pending = None
            if m_fc >= 0 and q > 0:
                fc_pending = fc_chunk_mms(m_fc, q - 1)
            xp_ps_pending = None
            if xp_m_next <= 15:
                if q == 0:
                    new_xp = xp_ring.tile([128, G4], bf16, tag="xp_m")
                    xp_tiles[xp_m_next] = new_xp
                xp_chunk_mms(xp_m_next, q, ks=(0, 1))

            # h = sigma(o) * tanh(c), halves; transpose each half into hsT
            nc.vector.tensor_mul(h_t[:, 0:256], nl[:, 1536:1792], tanhc[:, 0:256])
            hp = tp_psum.tile([128, 128], bf16, tag="pt")
            for k in (0, 1):
                nc.tensor.transpose(
                    hp[:, 32 * k : 32 * (k + 1)],
                    h_t[0:32, 128 * k : 128 * (k + 1)],
                    ident_b[0:32, 0:32],
                )
            nc.vector.tensor_copy(
                hsT[:, 0:2, 32 * t : 32 * (t + 1)],
                hp[:, 0:64].rearrange("p (k b) -> p k b", k=2),
            )
            nc.vector.tensor_mul(h_t[:, 256:512], nl[:, 1792:2048], tanhc[:, 256:512])
            hp2 = tp_psum.tile([128, 128], bf16, tag="pt")
            for k in (2, 3):
                nc.tensor.transpose(
                    hp2[:, 32 * (k - 2) : 32 * (k - 1)],
                    h_t[0:32, 128 * k : 128 * (k + 1)],
                    ident_b[0:32, 0:32],
                )
            nc.vector.tensor_copy(
                hsT[:, 2:4, 32 * t : 32 * (t + 1)],
                hp2[:, 0:64].rearrange("p (k b) -> p k b", k=2),
            )

            # --- remaining fillers / drains ---
            if xp_m_next <= 15:
                ps = xp_chunk_mms(xp_m_next, q, ks=(2, 3))
            if fc_pending is not None:
                fc_chunk_finish(m_fc, q - 1, fc_pending)
            if xp_m_next <= 15:
                xp_chunk_add(xp_m_next, q, ps)

            # --- preload next step's gate banks (scalar: g; vector: i,f,o)
            # emitted at the engines' queue tails, ordered so each bank is
            # refilled before step t+1's matmuls reach it ---
            if t + 1 < T:
                preload_scalar(t + 1, 0)
                preload_vector(t + 1, 1)

            # lazy xs transpose: tile t+1 (needed by produce_xp from step 4t)
            if 2 <= t + 1 <= 15:
                transpose_m(t + 1)

            if t + 1 < T:
                preload_vector(t + 1, 2)
                preload_vector(t + 1, 3)

        for j in range(3):
            fps = fc_chunk_mms(15, j)
            fc_chunk_finish(15, j, fps)

    nc.compile()
    return nc


def _get_program():
    global _PROGRAM
    if _PROGRAM is None:
        _PROGRAM = _build_program()
    return _PROGRAM


# PyTorch LSTM gate order is [i, f, g, o]; we reorder rows to [g, i, f, o] so
# the device-side column layout matches the compute order (tanh chunk first).
def _gate_perm():
    return np.concatenate(
        [np.arange(2 * H, 3 * H), np.arange(0, H), np.arange(H, 2 * H), np.arange(3 * H, 4 * H)]
    )


def _make_in_maps(features, captions, embed_table, W_ih, W_hh, b_ih, b_hh, fc_W, fc_b):
    import ml_dtypes

    bf16 = ml_dtypes.bfloat16
    perm = _gate_perm()
    features = np.ascontiguousarray(np.asarray(features, dtype=np.float32))
    cap = np.asarray(captions).astype(np.int32)                      # [B, T]
    embed = np.ascontiguousarray(np.asarray(embed_table, dtype=np.float32))
    wihT = np.ascontiguousarray(np.asarray(W_ih, dtype=np.float32)[perm].T.astype(bf16))
    whhT = np.ascontiguousarray(np.asarray(W_hh, dtype=np.float32)[perm].T.astype(bf16))
    bih = np.ascontiguousarray(np.asarray(b_ih, dtype=np.float32)[perm].astype(bf16))
    bhh = np.ascontiguousarray(np.asarray(b_hh, dtype=np.float32)[perm].astype(bf16))
    fc_W = np.asarray(fc_W, dtype=np.float32)
    fc_b = np.asarray(fc_b, dtype=np.float32)

    # gather indices, t-major: xs row t*32+b = embed[captions[b, t-1]] for t>=1
    idx = np.zeros(TB, dtype=np.int32)
    idx[B:] = cap[:, : T - 1].T.reshape(-1)

    in_maps = []
    for c in range(N_CORES):
        sl = slice(VSL * c, VSL * (c + 1))
        fcwT = np.zeros((H, VPAD), dtype=bf16)
        fcwT[:, :VSL] = fc_W[sl].T.astype(bf16)
        fcbp = np.zeros(VPAD, dtype=bf16)
        fcbp[:VSL] = fc_b[sl].astype(bf16)
        in_maps.append(
            dict(
                features=features,
                idx=idx,
                embed=embed,
                wihT=wihT,
                whhT=whhT,
                bih=bih,
                bhh=bhh,
                fcwT=np.ascontiguousarray(fcwT),
                fcb=fcbp,
                onesv=np.ones(128, dtype=bf16),
            )
        )
    return in_maps


def _install_ntff_hook():
    """Wire up NTFF profiling: bass_utils wants antenv.axon_hooks, which this
    container lacks; build it from trn_agent_boot's ctypes hook."""
    import sys as _sys
    import types

    if "antenv.axon_hooks" in _sys.modules:
        return
    if "/root/.axon_site" not in _sys.path:
        _sys.path.insert(0, "/root/.axon_site")
    from trn_agent_boot.trn_boot import _ntff_profile_via_ctypes

    hook = _ntff_profile_via_ctypes("/opt/axon/libaxon_pjrt.so")
    mod = types.ModuleType("antenv.axon_hooks")
    mod._hook = hook
    mod.set_axon_ntff_profile_hook = lambda h: setattr(mod, "_hook", h)
    mod.get_axon_ntff_profile_hook = lambda: mod._hook
    _sys.modules["antenv.axon_hooks"] = mod

    # avoid S3 uploads from the trace path in this zero-egress container
    import concourse.bass_utils as bu

    bu.upload_artifacts = lambda tmpdir: f"local:{tmpdir}"


def run(inputs, trace=False, trace_cores=None):
    """Run on hardware; returns (full_output [B,T,V] f32, BassKernelResults)."""
    from concourse.bass_utils import run_bass_kernel_spmd

    if trace:
        _install_ntff_hook()

    nc = _get_program()
    in_maps = _make_in_maps(
        inputs["features"],
        inputs["captions"],
        inputs["embed_table"],
        inputs["W_ih"],
        inputs["W_hh"],
        inputs["b_ih"],
        inputs["b_hh"],
        inputs["fc_W"],
        inputs["fc_b"],
    )
    kwargs = {}
    if trace:
        import os
        import shutil

        shutil.rmtree("/tmp/bass_trace", ignore_errors=True)
        os.makedirs("/tmp/bass_trace", exist_ok=True)
        kwargs.update(trace=True, trace_cores=trace_cores or [0], tmpdir="/tmp/bass_trace")
    res = run_bass_kernel_spmd(nc, in_maps, core_ids=list(range(N_CORES)), **kwargs)
    full = np.concatenate([r["out"] for r in res.results], axis=2)
    return full, res


def kernel(**inputs) -> np.ndarray:
    out, _ = run(inputs, trace=False)
    return out


# revision 14
# speedup vs baseline: 1.5072x; 1.5072x over previous
"""Trainium2 Bass kernel for nn_DecoderRNN (LSTM decoder + big vocab projection).

Strategy (8 NeuronCores, SPMD):
  - The LSTM recurrence (B=32, T=64, H=512) is replicated on every core:
    its per-step cost is dominated by streaming W_hh through the PE, which is
    batch-size independent, so sharding batch would not help, and sharding the
    gate dim across cores loses to per-step h-exchange latency.
  - The output projection (fc) is tensor-parallel over the vocab dim:
    core c computes logits[:, :, 1250*c : 1250*(c+1)] and writes its own
    [32, 64, 1250] output slab; the host concatenates slabs.
  - Embedding lookup runs on-device via indirect (gather) DMA, one tile per
    step so it never blocks the gpsimd engine's per-step work.
  - The input projection Xp = xs @ W_ih.T + b is computed in bulk (it has no
    recurrent dependency) into an SBUF ring, then PRELOADED into the gate
    PSUM banks by gpsimd one step ahead. The recurrent matmuls accumulate on
    top with start=False, so no PE cycles are spent injecting Xp.

Per-step pipeline (steady state ~5.5-6us):
  - 4 persistent PSUM banks hold the gates [32, 512] per chunk, column order
    [g | i | f | o] (host-permuted) so tanh(g), sigma(i), sigma(f) complete
    while the o matmuls still stream; o is computed in two 256-wide halves.
  - c = sigma(f)*c + sigma(i)*tanh(g) runs in two half-lanes: half 0 on the
    vector engine, half 1 on gpsimd, overlapped with the o matmuls.
  - After sigma(o): h = sigma(o)*tanh(c) per half, PE-transposed into hsT.
  - fc chunks and Xp production matmuls fill the PE while the elementwise
    tail of the step drains.

kernel(**inputs) takes FULL unsharded inputs, returns FULL [32, 64, 10000].
"""

import sys

sys.path.insert(0, "/opt/trn_rl_repo")

import numpy as np

N_CORES = 8
B, T = 32, 64
E, H, V = 512, 512, 10000
G4 = 4 * H            # 2048
TB = T * B            # 2048
VSL = V // N_CORES    # 1250 vocab rows per core
VPAD = 1280           # padded so fc N-chunks are 512/512/256 (all >=256)

_PROGRAM = None


def _build_program():
    import concourse.bass as bass
    import concourse.tile as tile
    from concourse import bacc, mybir
    from concourse.masks import make_identity
    from contextlib import ExitStack

    f32 = mybir.dt.float32
    bf16 = mybir.dt.bfloat16
    i32 = mybir.dt.int32
    AF = mybir.ActivationFunctionType

    nc = bacc.Bacc(
        "TRN2",
        target_bir_lowering=False,
        debug=False,
        num_devices=N_CORES,
    )

    features = nc.dram_tensor("features", [B, E], f32, kind="ExternalInput").ap()
    idx = nc.dram_tensor("idx", [TB], i32, kind="ExternalInput").ap()
    embed = nc.dram_tensor("embed", [V, E], f32, kind="ExternalInput").ap()
    wihT = nc.dram_tensor("wihT", [E, G4], bf16, kind="ExternalInput").ap()
    whhT = nc.dram_tensor("whhT", [H, G4], bf16, kind="ExternalInput").ap()
    bih = nc.dram_tensor("bih", [G4], bf16, kind="ExternalInput").ap()
    bhh = nc.dram_tensor("bhh", [G4], bf16, kind="ExternalInput").ap()
    fcwT = nc.dram_tensor("fcwT", [H, VPAD], bf16, kind="ExternalInput").ap()
    fcb = nc.dram_tensor("fcb", [VPAD], bf16, kind="ExternalInput").ap()
    onesv = nc.dram_tensor("onesv", [128], bf16, kind="ExternalInput").ap()
    out = nc.dram_tensor("out", [B, T, VSL], f32, kind="ExternalOutput").ap()
    # Output viewed as [t, b, v]: a 128-row t-major tb tile = 4 t planes.
    out_r = out.rearrange("b t v -> t b v")

    # gate column order (host permutes rows of W/bias to match):
    # [0:512) = g, [512:1024) = i, [1024:1536) = f, [1536:2048) = o
    CG, CI, CF, CO = 0, 1, 2, 3

    def col(c):
        return slice(512 * c, 512 * (c + 1))

    with tile.TileContext(nc) as tc, ExitStack() as ctx:
        # ---------------- persistent state ----------------
        state = ctx.enter_context(tc.tile_pool(name="state", bufs=1))
        # h.T history: block t holds h(t).T (written at the end of step t).
        # Layout [p, k, 32*t + b] = h(t)[b, 128*k + p]
        hsT = state.tile([128, 4, 32 * T], bf16, tag="hsT")
        whhT_sb = state.tile([128, 4, G4], bf16, tag="whhT")
        xsT = state.tile([128, 4, TB], bf16, tag="xsT")   # [p, k, tb] = xs[tb, 128k+p]
        wihT_sb = state.tile([128, 4, G4], bf16, tag="wihT")
        fcwT_sb = state.tile([128, 4, VPAD], bf16, tag="fcwT")
        fcb_sb = state.tile([1, VPAD], bf16, tag="fcb")
        bias128 = state.tile([128, G4], bf16, tag="bias128")
        fcb128 = state.tile([128, VPAD], f32, tag="fcb128")
        c_sb = state.tile([B, H], f32, tag="c")
        ident_b = state.tile([128, 128], bf16, tag="ident_b")
        ones = state.tile([1, 128], bf16, tag="ones")
        xs_b = state.tile([128, 16, E], bf16, tag="xs_b")
        idx_sb = state.tile([128, 16], i32, tag="idx")
        eye4 = state.tile([128, 32], bf16, tag="eye4")   # I32 stacked 4x

        # persistent PSUM: 4 gate banks, never start=True (preload overwrites)
        gstate = ctx.enter_context(tc.tile_pool(name="gstate", bufs=1, space="PSUM"))
        gb = []
        for c in range(4):
            gbank_c = gstate.tile([B, 512], f32, tag=f"g{c}", name=f"gbank{c}")
            gb.append(gbank_c)

        fc_psum = ctx.enter_context(tc.tile_pool(name="fc_ps", bufs=1, space="PSUM"))
        xp_psum = ctx.enter_context(tc.tile_pool(name="xp_ps", bufs=1, space="PSUM"))
        tp_psum = ctx.enter_context(tc.tile_pool(name="tp_ps", bufs=2, space="PSUM"))

        make_identity(nc, ident_b[:])
        for qq in range(4):
            nc.sync.dma_start(eye4[32 * qq : 32 * (qq + 1), :], ident_b[0:32, 0:32])
        nc.vector.memset(c_sb[:], 0.0)

        nc.sync.dma_start(fcb_sb[:], fcb[None, :])
        nc.sync.dma_start(ones[:], onesv[None, :])

        # ---------------- Xp production machinery ----------------
        # Produced m-tiles [128, 2048] live in an SBUF ring; the gpsimd
        # preload of step t copies rows [32*(t%4), +32) of tile t//4 into the
        # gate PSUM banks.
        xp_ring = ctx.enter_context(tc.tile_pool(name="xp_ring", bufs=2))
        xp_tiles = {}

        def xp_chunk_mms(m, cch, ks=(0, 1, 2, 3)):
            """PE part of one Xp chunk: psum += xs_tile @ W_ih.T[:, chunk]."""
            sl = col(cch)
            if ks[0] == 0:
                ps = xp_psum.tile([128, 512], f32, tag="xp")
                xp_chunk_mms.cur = ps
            else:
                ps = xp_chunk_mms.cur
            for k in ks:
                nc.tensor.matmul(
                    ps[:],
                    lhsT=xsT[:, k, 128 * m : 128 * (m + 1)],
                    rhs=wihT_sb[:, k, sl],
                    start=(k == 0),
                    stop=(k == 3),
                )
            return ps

        def xp_chunk_add(m, cch, ps):
            sl = col(cch)
            nc.vector.tensor_add(xp_tiles[m][:, sl], ps[:], bias128[:, sl])

        def produce_xp(m):
            xp_m_tile = xp_ring.tile([128, G4], bf16, tag="xp_m")
            xp_tiles[m] = xp_m_tile
            for cch in range(4):
                ps = xp_chunk_mms(m, cch)
                xp_chunk_add(m, cch, ps)

        # Xp injection into the gate banks: one eye4 matmul per chunk
        # (start=True opens the accumulation group; the whh matmuls pile on).
        # Emitted in the PE tail-gap of the previous step so it hides there.
        def inject(t, c, c0=0, csz=512):
            m, q = t // 4, t % 4
            nc.tensor.matmul(
                gb[c][:, c0 : c0 + csz],
                lhsT=eye4[32 * q : 32 * (q + 1), :],
                rhs=xp_tiles[m][32 * q : 32 * (q + 1), 512 * c + c0 : 512 * c + c0 + csz],
                start=True,
                stop=(t == 0),
                tile_position=(32 * q, 0),
            )

        def inject_all(t):
            for c in range(4):
                inject(t, c)

        # ---------------- embedding gather + xs.T transposes ----------------
        def gather(m):
            nc.gpsimd.indirect_dma_start(
                out=xs_b[:, m, :],
                out_offset=None,
                in_=embed[:, :],
                in_offset=bass.IndirectOffsetOnAxis(ap=idx_sb[:, m : m + 1], axis=0),
            )

        def transpose_m(m):
            for e in range(4):
                pt = tp_psum.tile([128, 128], bf16, tag="pt")
                nc.tensor.transpose(
                    pt[:], xs_b[:, m, 128 * e : 128 * (e + 1)], ident_b[:]
                )
                nc.vector.tensor_copy(xsT[:, e, 128 * m : 128 * (m + 1)], pt[:])

        # ---------------- prologue ----------------
        with ExitStack() as pro:
            small_pool = pro.enter_context(tc.tile_pool(name="small", bufs=1))
            bias1_sb = small_pool.tile([1, G4], bf16, tag="bias1")
            bias2_sb = small_pool.tile([1, G4], bf16, tag="bias2")

            nc.sync.dma_start(idx_sb[:], idx.rearrange("(m p) -> p m", p=128))
            nc.sync.dma_start(bias1_sb[:], bih[None, :])
            nc.sync.dma_start(bias2_sb[:], bhh[None, :])
            # wihT first (needed by the first Xp tiles), then whhT (step 1),
            # fcwT last (first used around step 4)
            nc.sync.dma_start(wihT_sb[:], wihT.rearrange("(k p) g -> p k g", p=128))
            nc.sync.dma_start(whhT_sb[:], whhT.rearrange("(k p) g -> p k g", p=128))
            nc.sync.dma_start(fcwT_sb[:], fcwT.rearrange("(k p) v -> p k v", p=128))

            # bias128 = broadcast(b_ih + b_hh) via rank-1 matmuls
            for cch in range(4):
                sl = col(cch)
                bp = xp_psum.tile([128, 512], f32, tag="xp")
                nc.tensor.matmul(bp[:], lhsT=ones[0:1, :], rhs=bias1_sb[0:1, sl],
                                 start=True, stop=False)
                nc.tensor.matmul(bp[:], lhsT=ones[0:1, :], rhs=bias2_sb[0:1, sl],
                                 start=False, stop=True)
                nc.vector.tensor_copy(bias128[:, sl], bp[:])
            # fcb128 = broadcast(fc_b) via rank-1 matmuls
            for c0, csz in ((0, 512), (512, 512), (1024, 256)):
                bp = xp_psum.tile([128, 512], f32, tag="xp")
                nc.tensor.matmul(bp[:, 0:csz], lhsT=ones[0:1, :],
                                 rhs=fcb_sb[0:1, c0 : c0 + csz], start=True, stop=True)
                nc.vector.tensor_copy(fcb128[:, c0 : c0 + csz], bp[:, 0:csz])

            # first xs tiles: gather 0 (covers steps 1..4 inputs), features
            # overwrite rows 0..31 (step 0 input)
            gather(0)
            nc.gpsimd.dma_start(xs_b[0:32, 0, :], features[:, :])
            gather(1)
            transpose_m(0)
            produce_xp(0)
            inject_all(0)
            transpose_m(1)

        # ---------------- main recurrence + interleaved fc/Xp ----------------
        work = ctx.enter_context(tc.tile_pool(name="work", bufs=2))
        lg_pool = ctx.enter_context(tc.tile_pool(name="lg", bufs=2))

        FC_CHUNKS = ((0, 512), (512, 512), (1024, 256))
        lg_tiles = {}

        def fc_chunk_mms(m, j):
            if j == 0:
                lg_new = lg_pool.tile([128, VPAD], f32, tag="lg")
                lg_tiles[m] = lg_new
            c0, csz = FC_CHUNKS[j]
            fps = fc_psum.tile([128, 512], f32, tag="fc")
            for k in range(4):
                nc.tensor.matmul(
                    fps[:, 0:csz],
                    lhsT=hsT[:, k, 128 * m : 128 * (m + 1)],
                    rhs=fcwT_sb[:, k, c0 : c0 + csz],
                    start=(k == 0),
                    stop=(k == 3),
                )
            return fps

        def fc_chunk_finish(m, j, fps):
            c0, csz = FC_CHUNKS[j]
            nc.vector.tensor_add(
                lg_tiles[m][:, c0 : c0 + csz], fps[:, 0:csz], fcb128[:, c0 : c0 + csz]
            )
            if j == 2:
                nc.sync.dma_start(
                    out_r[4 * m : 4 * (m + 1), :, :], lg_tiles[m][:, 0:VSL]
                )

        for t in range(T):
            q = t % 4
            xp_m_next = t // 4 + 1

            # per-gate activation outputs in separate SBUF tiles (avoids DVE
            # same-tile dual-read port conflicts); bf16 so all-16-bit vector
            # ops run at 2x. c stays f32 for accuracy.
            ng = work.tile([B, H], bf16, tag="ng")
            ni = work.tile([B, H], bf16, tag="ni")
            nf = work.tile([B, H], bf16, tag="nf")
            no = work.tile([B, H], bf16, tag="no")
            tanhc = work.tile([B, H], bf16, tag="tanhc")
            fmul = work.tile([B, H], f32, tag="fmul")
            ig = work.tile([B, H], bf16, tag="ig")
            h_t = work.tile([B, H], bf16, tag="h")

            def gate_mms(cch, c0=0, csz=512, close=True):
                if t == 0:
                    return  # gates(0) = Xp(0) injected in the prologue
                for k in range(4):
                    nc.tensor.matmul(
                        gb[cch][:, c0 : c0 + csz],
                        lhsT=hsT[:, k, 32 * (t - 1) : 32 * t],
                        rhs=whhT_sb[:, k, 512 * cch + c0 : 512 * cch + c0 + csz],
                        start=False,
                        stop=(k == 3 and close),
                    )

            # --- PE: g, i, f chunk matmuls; scalar acts interleaved ---
            gate_mms(CG)
            # lazy gather early in the gpsimd queue (engine idle here):
            # tile t+2 gathered at step t, transposed at step t+1
            if t + 2 <= 15:
                gather(t + 2)
            nc.scalar.activation(ng[:], gb[CG][:], AF.Tanh)
            gate_mms(CI)
            nc.scalar.activation(ni[:], gb[CI][:], AF.Sigmoid)
            # half-lanes: ig = tanh(g)*sigma(i); vector half0, gpsimd half1
            nc.vector.tensor_mul(ig[:, 0:256], ng[:, 0:256], ni[:, 0:256])
            nc.gpsimd.tensor_mul(ig[:, 256:512], ng[:, 256:512], ni[:, 256:512])
            gate_mms(CF)
            nc.scalar.activation(nf[:], gb[CF][:], AF.Sigmoid)
            nc.vector.tensor_mul(fmul[:, 0:256], nf[:, 0:256], c_sb[:, 0:256])
            nc.gpsimd.tensor_mul(fmul[:, 256:512], nf[:, 256:512], c_sb[:, 256:512])
            # --- o chunk (single group: a PSUM bank allows only one
            # accumulation group and blocks reads until it closes) ---
            gate_mms(CO)
            nc.scalar.activation(no[:], gb[CO][:], AF.Sigmoid)

            # c = fmul + ig (both halves on vector; gpsimd adds are slow),
            # tanh(c) interleaved on scalar
            nc.vector.tensor_add(c_sb[:, 0:256], fmul[:, 0:256], ig[:, 0:256])
            nc.scalar.activation(tanhc[:, 0:256], c_sb[:, 0:256], AF.Tanh)
            nc.vector.tensor_add(c_sb[:, 256:512], fmul[:, 256:512], ig[:, 256:512])
            nc.scalar.activation(tanhc[:, 256:512], c_sb[:, 256:512], AF.Tanh)

            # --- PE fillers while the tail drains: fc chunk + xp production ---
            m_fc = t // 4 - 1
            fc_pending = None
            if m_fc >= 0 and q > 0:
                fc_pending = fc_chunk_mms(m_fc, q - 1)
            xp_ps_pending = None
            if xp_m_next <= 15:
                if q == 0:
                    new_xp = xp_ring.tile([128, G4], bf16, tag="xp_m")
                    xp_tiles[xp_m_next] = new_xp
                xp_chunk_mms(xp_m_next, q, ks=(0, 1))

            # h = sigma(o) * tanh(c), halves; transpose each half into hsT
            nc.vector.tensor_mul(h_t[:, 0:256], no[:, 0:256], tanhc[:, 0:256])
            hp = tp_psum.tile([128, 128], bf16, tag="pt")
            for k in (0, 1):
                nc.tensor.transpose(
                    hp[:, 32 * k : 32 * (k + 1)],
                    h_t[0:32, 128 * k : 128 * (k + 1)],
                    ident_b[0:32, 0:32],
                )
            nc.vector.tensor_copy(
                hsT[:, 0:2, 32 * t : 32 * (t + 1)],
                hp[:, 0:64].rearrange("p (k b) -> p k b", k=2),
            )
            nc.vector.tensor_mul(h_t[:, 256:512], no[:, 256:512], tanhc[:, 256:512])
            hp2 = tp_psum.tile([128, 128], bf16, tag="pt")
            for k in (2, 3):
                nc.tensor.transpose(
                    hp2[:, 32 * (k - 2) : 32 * (k - 1)],
                    h_t[0:32, 128 * k : 128 * (k + 1)],
                    ident_b[0:32, 0:32],
                )
            nc.vector.tensor_copy(
                hsT[:, 2:4, 32 * t : 32 * (t + 1)],
                hp2[:, 0:64].rearrange("p (k b) -> p k b", k=2),
            )

            # --- remaining fillers / drains; next step's injects in the tail.
            # inject(t+1, c) reads ring columns col(c), which at q==3 for c=3
            # are drained only by this step's xp_chunk_add -> emit inject-o
            # after the add.
            if xp_m_next <= 15:
                ps = xp_chunk_mms(xp_m_next, q, ks=(2, 3))
            if t + 1 < T:
                inject(t + 1, 0)
                inject(t + 1, 1)
                inject(t + 1, 2)
            if fc_pending is not None:
                fc_chunk_finish(m_fc, q - 1, fc_pending)
            if xp_m_next <= 15:
                xp_chunk_add(xp_m_next, q, ps)
            if t + 1 < T:
                inject(t + 1, 3)

            # lazy xs transpose: tile t+1 (needed by produce_xp from step 4t)
            if 2 <= t + 1 <= 15:
                transpose_m(t + 1)

        for j in range(3):
            fps = fc_chunk_mms(15, j)
            fc_chunk_finish(15, j, fps)

    nc.compile()
    return nc


def _get_program():
    global _PROGRAM
    if _PROGRAM is None:
        _PROGRAM = _build_program()
    return _PROGRAM


# PyTorch LSTM gate order is [i, f, g, o]; we reorder rows to [g, i, f, o] so
# the device-side column layout matches the compute order (tanh chunk first).
def _gate_perm():
    return np.concatenate(
        [np.arange(2 * H, 3 * H), np.arange(0, H), np.arange(H, 2 * H), np.arange(3 * H, 4 * H)]
    )


def _make_in_maps(features, captions, embed_table, W_ih, W_hh, b_ih, b_hh, fc_W, fc_b):
    import ml_dtypes

    bf16 = ml_dtypes.bfloat16
    perm = _gate_perm()
    features = np.ascontiguousarray(np.asarray(features, dtype=np.float32))
    cap = np.asarray(captions).astype(np.int32)                      # [B, T]
    embed = np.ascontiguousarray(np.asarray(embed_table, dtype=np.float32))
    wihT = np.ascontiguousarray(np.asarray(W_ih, dtype=np.float32)[perm].T.astype(bf16))
    whhT = np.ascontiguousarray(np.asarray(W_hh, dtype=np.float32)[perm].T.astype(bf16))
    bih = np.ascontiguousarray(np.asarray(b_ih, dtype=np.float32)[perm].astype(bf16))
    bhh = np.ascontiguousarray(np.asarray(b_hh, dtype=np.float32)[perm].astype(bf16))
    fc_W = np.asarray(fc_W, dtype=np.float32)
    fc_b = np.asarray(fc_b, dtype=np.float32)

    # gather indices, t-major: xs row t*32+b = embed[captions[b, t-1]] for t>=1
    idx = np.zeros(TB, dtype=np.int32)
    idx[B:] = cap[:, : T - 1].T.reshape(-1)

    in_maps = []
    for c in range(N_CORES):
        sl = slice(VSL * c, VSL * (c + 1))
        fcwT = np.zeros((H, VPAD), dtype=bf16)
        fcwT[:, :VSL] = fc_W[sl].T.astype(bf16)
        fcbp = np.zeros(VPAD, dtype=bf16)
        fcbp[:VSL] = fc_b[sl].astype(bf16)
        in_maps.append(
            dict(
                features=features,
                idx=idx,
                embed=embed,
                wihT=wihT,
                whhT=whhT,
                bih=bih,
                bhh=bhh,
                fcwT=np.ascontiguousarray(fcwT),
                fcb=fcbp,
                onesv=np.ones(128, dtype=bf16),
            )
        )
    return in_maps


def _install_ntff_hook():
    """Wire up NTFF profiling: bass_utils wants antenv.axon_hooks, which this
    container lacks; build it from trn_agent_boot's ctypes hook."""
    import sys as _sys
    import types

    if "antenv.axon_hooks" in _sys.modules:
        return
    if "/root/.axon_site" not in _sys.path:
        _sys.path.insert(0, "/root/.axon_site")
    from trn_agent_boot.trn_boot import _ntff_profile_via_ctypes

    hook = _ntff_profile_via_ctypes("/opt/axon/libaxon_pjrt.so")
    mod = types.ModuleType("antenv.axon_hooks")
    mod._hook = hook
    mod.set_axon_ntff_profile_hook = lambda h: setattr(mod, "_hook", h)
    mod.get_axon_ntff_profile_hook = lambda: mod._hook
    _sys.modules["antenv.axon_hooks"] = mod

    # avoid S3 uploads from the trace path in this zero-egress container
    import concourse.bass_utils as bu

    bu.upload_artifacts = lambda tmpdir: f"local:{tmpdir}"


def run(inputs, trace=False, trace_cores=None):
    """Run on hardware; returns (full_output [B,T,V] f32, BassKernelResults)."""
    from concourse.bass_utils import run_bass_kernel_spmd

    if trace:
        _install_ntff_hook()

    nc = _get_program()
    in_maps = _make_in_maps(
        inputs["features"],
        inputs["captions"],
        inputs["embed_table"],
        inputs["W_ih"],
        inputs["W_hh"],
        inputs["b_ih"],
        inputs["b_hh"],
        inputs["fc_W"],
        inputs["fc_b"],
    )
    kwargs = {}
    if trace:
        import os
        import shutil

        shutil.rmtree("/tmp/bass_trace", ignore_errors=True)
        os.makedirs("/tmp/bass_trace", exist_ok=True)
        kwargs.update(trace=True, trace_cores=trace_cores or [0], tmpdir="/tmp/bass_trace")
    res = run_bass_kernel_spmd(nc, in_maps, core_ids=list(range(N_CORES)), **kwargs)
    full = np.concatenate([r["out"] for r in res.results], axis=2)
    return full, res


def kernel(**inputs) -> np.ndarray:
    out, _ = run(inputs, trace=False)
    return out


# revision 16
# speedup vs baseline: 1.7320x; 1.1492x over previous
"""Trainium2 Bass kernel for nn_DecoderRNN (LSTM decoder + big vocab projection).

Strategy (8 NeuronCores, SPMD):
  - The LSTM recurrence (B=32, T=64, H=512) is replicated on every core:
    its per-step cost is dominated by streaming W_hh through the PE, which is
    batch-size independent, so sharding batch would not help, and sharding the
    gate dim across cores loses to per-step h-exchange latency.
  - The output projection (fc) is tensor-parallel over the vocab dim:
    core c computes logits[:, :, 1250*c : 1250*(c+1)] and writes its own
    [32, 64, 1250] output slab; the host concatenates slabs.
  - Embedding lookup runs on-device via indirect (gather) DMA, one tile per
    step so it never blocks the gpsimd engine's per-step work.
  - The input projection Xp = xs @ W_ih.T + b is computed in bulk (it has no
    recurrent dependency) into an SBUF ring, then PRELOADED into the gate
    PSUM banks by gpsimd one step ahead. The recurrent matmuls accumulate on
    top with start=False, so no PE cycles are spent injecting Xp.

Per-step pipeline (steady state ~5.5-6us):
  - 4 persistent PSUM banks hold the gates [32, 512] per chunk, column order
    [g | i | f | o] (host-permuted) so tanh(g), sigma(i), sigma(f) complete
    while the o matmuls still stream; o is computed in two 256-wide halves.
  - c = sigma(f)*c + sigma(i)*tanh(g) runs in two half-lanes: half 0 on the
    vector engine, half 1 on gpsimd, overlapped with the o matmuls.
  - After sigma(o): h = sigma(o)*tanh(c) per half, PE-transposed into hsT.
  - fc chunks and Xp production matmuls fill the PE while the elementwise
    tail of the step drains.

kernel(**inputs) takes FULL unsharded inputs, returns FULL [32, 64, 10000].
"""

import sys

sys.path.insert(0, "/opt/trn_rl_repo")

import numpy as np

N_CORES = 8
B, T = 32, 64
E, H, V = 512, 512, 10000
G4 = 4 * H            # 2048
TB = T * B            # 2048
VSL = V // N_CORES    # 1250 vocab rows per core
VPAD = 1280           # padded so fc N-chunks are 512/512/256 (all >=256)

_PROGRAM = None


def _build_program():
    import concourse.bass as bass
    import concourse.tile as tile
    from concourse import bacc, mybir
    from concourse.masks import make_identity
    from contextlib import ExitStack

    f32 = mybir.dt.float32
    bf16 = mybir.dt.bfloat16
    i32 = mybir.dt.int32
    AF = mybir.ActivationFunctionType

    nc = bacc.Bacc(
        "TRN2",
        target_bir_lowering=False,
        debug=False,
        num_devices=N_CORES,
    )

    features = nc.dram_tensor("features", [B, E], f32, kind="ExternalInput").ap()
    idx = nc.dram_tensor("idx", [TB], i32, kind="ExternalInput").ap()
    embed = nc.dram_tensor("embed", [V, E], f32, kind="ExternalInput").ap()
    wihT = nc.dram_tensor("wihT", [E, G4], bf16, kind="ExternalInput").ap()
    whhT = nc.dram_tensor("whhT", [H, G4], bf16, kind="ExternalInput").ap()
    bsum = nc.dram_tensor("bsum", [G4], bf16, kind="ExternalInput").ap()
    fcwT = nc.dram_tensor("fcwT", [H, VPAD], bf16, kind="ExternalInput").ap()
    fcb = nc.dram_tensor("fcb", [VPAD], bf16, kind="ExternalInput").ap()
    onesv = nc.dram_tensor("onesv", [128], bf16, kind="ExternalInput").ap()
    out = nc.dram_tensor("out", [B, T, VSL], bf16, kind="ExternalOutput").ap()
    # Output viewed as [t, b, v]: a 128-row t-major tb tile = 4 t planes.
    out_r = out.rearrange("b t v -> t b v")

    # gate column order (host permutes rows of W/bias to match):
    # [0:512) = g, [512:1024) = i, [1024:1536) = f, [1536:2048) = o
    CG, CI, CF, CO = 0, 1, 2, 3

    def col(c):
        return slice(512 * c, 512 * (c + 1))

    with tile.TileContext(nc) as tc, ExitStack() as ctx:
        # ---------------- persistent state ----------------
        state = ctx.enter_context(tc.tile_pool(name="state", bufs=1))
        # h.T history: block t holds h(t).T (written at the end of step t).
        # Layout [p, k, 32*t + b] = h(t)[b, 128*k + p]
        hsT = state.tile([128, 4, 32 * T], bf16, tag="hsT")
        whhT_sb = state.tile([128, 4, G4], bf16, tag="whhT")
        xsT = state.tile([128, 4, TB], bf16, tag="xsT")   # [p, k, tb] = xs[tb, 128k+p]
        wihT_sb = state.tile([128, 4, G4], bf16, tag="wihT")
        fcwT_sb = state.tile([128, 4, VPAD], bf16, tag="fcwT")
        fcb_sb = state.tile([1, VPAD], bf16, tag="fcb")
        bsum_sb = state.tile([1, G4], bf16, tag="bsum")
        c_sb = state.tile([B, H], bf16, tag="c")
        ident_b = state.tile([128, 128], bf16, tag="ident_b")
        ones = state.tile([1, 128], bf16, tag="ones")
        xs_b = state.tile([128, 16, E], bf16, tag="xs_b")
        idx_sb = state.tile([128, 16], i32, tag="idx")
        eye4 = state.tile([128, 32], bf16, tag="eye4")   # I32 stacked 4x

        # persistent PSUM: 4 gate banks, never start=True (preload overwrites)
        gstate = ctx.enter_context(tc.tile_pool(name="gstate", bufs=1, space="PSUM"))
        gb = []
        for c in range(4):
            gbank_c = gstate.tile([B, 512], f32, tag=f"g{c}", name=f"gbank{c}")
            gb.append(gbank_c)

        fc_psum = ctx.enter_context(tc.tile_pool(name="fc_ps", bufs=1, space="PSUM"))
        xp_psum = ctx.enter_context(tc.tile_pool(name="xp_ps", bufs=2, space="PSUM"))
        tp_psum = ctx.enter_context(tc.tile_pool(name="tp_ps", bufs=1, space="PSUM"))

        make_identity(nc, ident_b[:])
        for qq in range(4):
            nc.sync.dma_start(eye4[32 * qq : 32 * (qq + 1), :], ident_b[0:32, 0:32])
        nc.vector.memset(c_sb[:], 0.0)

        nc.sync.dma_start(fcb_sb[:], fcb[None, :])
        nc.sync.dma_start(ones[:], onesv[None, :])

        # ---------------- Xp production machinery ----------------
        # Produced m-tiles [128, 2048] live in an SBUF ring; the gpsimd
        # preload of step t copies rows [32*(t%4), +32) of tile t//4 into the
        # gate PSUM banks.
        xp_ring = ctx.enter_context(tc.tile_pool(name="xp_ring", bufs=2))
        xp_tiles = {}

        def xp_chunk_mms(m, cch, ks=(0, 1, 2, 3)):
            """PE part of one Xp chunk: psum = xs_tile @ W_ih.T[:, chunk] + b
            (bias folded in as a rank-1 ones x bsum matmul)."""
            sl = col(cch)
            if ks[0] == 0:
                ps = xp_psum.tile([128, 512], f32, tag="xp")
                xp_chunk_mms.cur = ps
            else:
                ps = xp_chunk_mms.cur
            for k in ks:
                nc.tensor.matmul(
                    ps[:],
                    lhsT=xsT[:, k, 128 * m : 128 * (m + 1)],
                    rhs=wihT_sb[:, k, sl],
                    start=(k == 0),
                    stop=False,
                )
            if ks[-1] == 3:
                nc.tensor.matmul(ps[:], lhsT=ones[0:1, :], rhs=bsum_sb[0:1, sl],
                                 start=False, stop=True)
            return ps

        def xp_chunk_add(m, cch, ps):
            sl = col(cch)
            nc.scalar.copy(xp_tiles[m][:, sl], ps[:])

        def produce_xp(m):
            xp_m_tile = xp_ring.tile([128, G4], bf16, tag="xp_m")
            xp_tiles[m] = xp_m_tile
            for cch in range(4):
                ps = xp_chunk_mms(m, cch)
                xp_chunk_add(m, cch, ps)

        # Xp injection into the gate banks: one eye4 matmul per chunk
        # (start=True opens the accumulation group; the whh matmuls pile on).
        # Emitted in the PE tail-gap of the previous step so it hides there.
        def inject(t, c, c0=0, csz=512):
            m, q = t // 4, t % 4
            nc.tensor.matmul(
                gb[c][:, c0 : c0 + csz],
                lhsT=eye4[32 * q : 32 * (q + 1), :],
                rhs=xp_tiles[m][32 * q : 32 * (q + 1), 512 * c + c0 : 512 * c + c0 + csz],
                start=True,
                stop=(t == 0),
                tile_position=(32 * q, 0),
            )

        def inject_all(t):
            for c in range(4):
                inject(t, c)

        # ---------------- embedding gather + xs.T transposes ----------------
        def gather(m):
            nc.gpsimd.indirect_dma_start(
                out=xs_b[:, m, :],
                out_offset=None,
                in_=embed[:, :],
                in_offset=bass.IndirectOffsetOnAxis(ap=idx_sb[:, m : m + 1], axis=0),
            )

        def transpose_m(m):
            for e in range(4):
                pt = tp_psum.tile([128, 128], bf16, tag="pt")
                nc.tensor.transpose(
                    pt[:], xs_b[:, m, 128 * e : 128 * (e + 1)], ident_b[:]
                )
                nc.vector.tensor_copy(xsT[:, e, 128 * m : 128 * (m + 1)], pt[:])

        # ---------------- prologue ----------------
        with ExitStack() as pro:
            nc.sync.dma_start(idx_sb[:], idx.rearrange("(m p) -> p m", p=128))
            nc.sync.dma_start(bsum_sb[:], bsum[None, :])
            # weight loads split per-k so the compiler spreads them over many
            # DMA queues in parallel (a single 2MB DMA runs ~90us on one ring)
            wihT_r = wihT.rearrange("(k p) g -> k p g", p=128)
            whhT_r = whhT.rearrange("(k p) g -> k p g", p=128)
            fcwT_r = fcwT.rearrange("(k p) v -> k p v", p=128)
            for k in range(4):
                nc.sync.dma_start(wihT_sb[:, k, 0:1024], wihT_r[k, :, 0:1024])
                nc.sync.dma_start(wihT_sb[:, k, 1024:2048], wihT_r[k, :, 1024:2048])
            for k in range(4):
                nc.sync.dma_start(whhT_sb[:, k, 0:1024], whhT_r[k, :, 0:1024])
                nc.sync.dma_start(whhT_sb[:, k, 1024:2048], whhT_r[k, :, 1024:2048])
            for k in range(4):
                nc.sync.dma_start(fcwT_sb[:, k, :], fcwT_r[k, :, :])

            # first xs tiles: gather 0 (covers steps 1..4 inputs), features
            # overwrite rows 0..31 (step 0 input)
            gather(0)
            nc.gpsimd.dma_start(xs_b[0:32, 0, :], features[:, :])
            gather(1)
            transpose_m(0)
            produce_xp(0)
            inject_all(0)
            transpose_m(1)

        # ---------------- main recurrence + interleaved fc/Xp ----------------
        work = ctx.enter_context(tc.tile_pool(name="work", bufs=2))
        lg_pool = ctx.enter_context(tc.tile_pool(name="lg", bufs=2))

        FC_CHUNKS = ((0, 512), (512, 512), (1024, 256))
        lg_tiles = {}

        def fc_chunk_mms(m, j):
            if j == 0:
                lg_new = lg_pool.tile([128, VPAD], bf16, tag="lg")
                lg_tiles[m] = lg_new
            c0, csz = FC_CHUNKS[j]
            fps = fc_psum.tile([128, 512], f32, tag="fc")
            for k in range(4):
                nc.tensor.matmul(
                    fps[:, 0:csz],
                    lhsT=hsT[:, k, 128 * m : 128 * (m + 1)],
                    rhs=fcwT_sb[:, k, c0 : c0 + csz],
                    start=(k == 0),
                    stop=False,
                )
            nc.tensor.matmul(fps[:, 0:csz], lhsT=ones[0:1, :],
                             rhs=fcb_sb[0:1, c0 : c0 + csz], start=False, stop=True)
            return fps

        def fc_chunk_finish(m, j, fps):
            c0, csz = FC_CHUNKS[j]
            nc.scalar.copy(lg_tiles[m][:, c0 : c0 + csz], fps[:, 0:csz])
            if j == 2:
                nc.sync.dma_start(
                    out_r[4 * m : 4 * (m + 1), :, :], lg_tiles[m][:, 0:VSL]
                )

        for t in range(T):
            q = t % 4
            xp_m_next = t // 4 + 1

            # per-gate activation outputs in separate SBUF tiles (avoids DVE
            # same-tile dual-read port conflicts); bf16 so all-16-bit vector
            # ops run at 2x. c stays f32 for accuracy.
            ng = work.tile([B, H], bf16, tag="ng")
            ni = work.tile([B, H], bf16, tag="ni")
            nf = work.tile([B, H], bf16, tag="nf")
            no = work.tile([B, H], bf16, tag="no")
            tanhc = work.tile([B, H], bf16, tag="tanhc")
            fmul = work.tile([B, H], bf16, tag="fmul")
            ig = work.tile([B, H], bf16, tag="ig")
            h_t = work.tile([B, H], bf16, tag="h")

            def gate_mms(cch, c0=0, csz=512, close=True):
                if t == 0:
                    return  # gates(0) = Xp(0) injected in the prologue
                for k in range(4):
                    nc.tensor.matmul(
                        gb[cch][:, c0 : c0 + csz],
                        lhsT=hsT[:, k, 32 * (t - 1) : 32 * t],
                        rhs=whhT_sb[:, k, 512 * cch + c0 : 512 * cch + c0 + csz],
                        start=False,
                        stop=(k == 3 and close),
                    )

            # --- PE: g, i, f chunk matmuls; scalar acts interleaved ---
            gate_mms(CG)
            # lazy gather early in the gpsimd queue (engine idle here):
            # tile t+2 gathered at step t, transposed at step t+1
            if t + 2 <= 15:
                gather(t + 2)
            nc.scalar.activation(ng[:], gb[CG][:], AF.Tanh)
            gate_mms(CI)
            nc.scalar.activation(ni[:], gb[CI][:], AF.Sigmoid)
            # all-bf16 vector chain (DVE 16-bit runs 2x; gpsimd is slow)
            nc.vector.tensor_mul(ig[:], ng[:], ni[:])
            gate_mms(CF)
            nc.scalar.activation(nf[:], gb[CF][:], AF.Sigmoid)
            nc.vector.tensor_mul(fmul[:], nf[:], c_sb[:])
            # --- o chunk (single group: a PSUM bank allows only one
            # accumulation group and blocks reads until it closes) ---
            gate_mms(CO)
            nc.scalar.activation(no[:], gb[CO][:], AF.Sigmoid)

            # c = fmul + ig in halves (earlier tanh(c0) start on scalar)
            nc.vector.tensor_add(c_sb[:, 0:256], fmul[:, 0:256], ig[:, 0:256])
            nc.scalar.activation(tanhc[:, 0:256], c_sb[:, 0:256], AF.Tanh)
            nc.vector.tensor_add(c_sb[:, 256:512], fmul[:, 256:512], ig[:, 256:512])
            nc.scalar.activation(tanhc[:, 256:512], c_sb[:, 256:512], AF.Tanh)

            # --- PE fillers while the tail drains: fc chunk + xp production ---
            m_fc = t // 4 - 1
            fc_pending = None
            if m_fc >= 0 and q > 0:
                fc_pending = fc_chunk_mms(m_fc, q - 1)
            xp_ps_pending = None
            if xp_m_next <= 15:
                if q == 0:
                    new_xp = xp_ring.tile([128, G4], bf16, tag="xp_m")
                    xp_tiles[xp_m_next] = new_xp
                xp_chunk_mms(xp_m_next, q, ks=(0, 1))

            # h = sigma(o) * tanh(c), halves; transpose each half into hsT
            nc.vector.tensor_mul(h_t[:, 0:256], no[:, 0:256], tanhc[:, 0:256])
            hp = tp_psum.tile([128, 128], bf16, tag="pt")
            for k in (0, 1):
                nc.tensor.transpose(
                    hp[:, 32 * k : 32 * (k + 1)],
                    h_t[0:32, 128 * k : 128 * (k + 1)],
                    ident_b[0:32, 0:32],
                )
            nc.vector.tensor_copy(
                hsT[:, 0:2, 32 * t : 32 * (t + 1)],
                hp[:, 0:64].rearrange("p (k b) -> p k b", k=2),
            )
            nc.vector.tensor_mul(h_t[:, 256:512], no[:, 256:512], tanhc[:, 256:512])
            hp2 = tp_psum.tile([128, 128], bf16, tag="pt")
            for k in (2, 3):
                nc.tensor.transpose(
                    hp2[:, 32 * (k - 2) : 32 * (k - 1)],
                    h_t[0:32, 128 * k : 128 * (k + 1)],
                    ident_b[0:32, 0:32],
                )
            nc.vector.tensor_copy(
                hsT[:, 2:4, 32 * t : 32 * (t + 1)],
                hp2[:, 0:64].rearrange("p (k b) -> p k b", k=2),
            )

            # --- remaining fillers / drains; next step's injects in the tail.
            # inject(t+1, c) reads ring columns col(c), which at q==3 for c=3
            # are drained only by this step's xp_chunk_add -> emit inject-o
            # after the add.
            if xp_m_next <= 15:
                ps = xp_chunk_mms(xp_m_next, q, ks=(2, 3))
            if t + 1 < T:
                inject(t + 1, 0)
                inject(t + 1, 1)
                inject(t + 1, 2)
            if fc_pending is not None:
                fc_chunk_finish(m_fc, q - 1, fc_pending)
            if xp_m_next <= 15:
                xp_chunk_add(xp_m_next, q, ps)
            if t + 1 < T:
                inject(t + 1, 3)

            # lazy xs transpose: tile t+1 (needed by produce_xp from step 4t)
            if 2 <= t + 1 <= 15:
                transpose_m(t + 1)

        for j in range(3):
            fps = fc_chunk_mms(15, j)
            fc_chunk_finish(15, j, fps)

    nc.compile()
    return nc


def _get_program():
    global _PROGRAM
    if _PROGRAM is None:
        _PROGRAM = _build_program()
    return _PROGRAM


# PyTorch LSTM gate order is [i, f, g, o]; we reorder rows to [g, i, f, o] so
# the device-side column layout matches the compute order (tanh chunk first).
def _gate_perm():
    return np.concatenate(
        [np.arange(2 * H, 3 * H), np.arange(0, H), np.arange(H, 2 * H), np.arange(3 * H, 4 * H)]
    )


def _make_in_maps(features, captions, embed_table, W_ih, W_hh, b_ih, b_hh, fc_W, fc_b):
    import ml_dtypes

    bf16 = ml_dtypes.bfloat16
    perm = _gate_perm()
    features = np.ascontiguousarray(np.asarray(features, dtype=np.float32))
    cap = np.asarray(captions).astype(np.int32)                      # [B, T]
    embed = np.ascontiguousarray(np.asarray(embed_table, dtype=np.float32))
    wihT = np.ascontiguousarray(np.asarray(W_ih, dtype=np.float32)[perm].T.astype(bf16))
    whhT = np.ascontiguousarray(np.asarray(W_hh, dtype=np.float32)[perm].T.astype(bf16))
    bsum = np.ascontiguousarray(
        (np.asarray(b_ih, dtype=np.float32) + np.asarray(b_hh, dtype=np.float32))[perm].astype(bf16)
    )
    fc_W = np.asarray(fc_W, dtype=np.float32)
    fc_b = np.asarray(fc_b, dtype=np.float32)

    # gather indices, t-major: xs row t*32+b = embed[captions[b, t-1]] for t>=1
    idx = np.zeros(TB, dtype=np.int32)
    idx[B:] = cap[:, : T - 1].T.reshape(-1)

    in_maps = []
    for c in range(N_CORES):
        sl = slice(VSL * c, VSL * (c + 1))
        fcwT = np.zeros((H, VPAD), dtype=bf16)
        fcwT[:, :VSL] = fc_W[sl].T.astype(bf16)
        fcbp = np.zeros(VPAD, dtype=bf16)
        fcbp[:VSL] = fc_b[sl].astype(bf16)
        in_maps.append(
            dict(
                features=features,
                idx=idx,
                embed=embed,
                wihT=wihT,
                whhT=whhT,
                bsum=bsum,
                fcwT=np.ascontiguousarray(fcwT),
                fcb=fcbp,
                onesv=np.ones(128, dtype=bf16),
            )
        )
    return in_maps


def _install_ntff_hook():
    """Wire up NTFF profiling: bass_utils wants antenv.axon_hooks, which this
    container lacks; build it from trn_agent_boot's ctypes hook."""
    import sys as _sys
    import types

    if "antenv.axon_hooks" in _sys.modules:
        return
    if "/root/.axon_site" not in _sys.path:
        _sys.path.insert(0, "/root/.axon_site")
    from trn_agent_boot.trn_boot import _ntff_profile_via_ctypes

    hook = _ntff_profile_via_ctypes("/opt/axon/libaxon_pjrt.so")
    mod = types.ModuleType("antenv.axon_hooks")
    mod._hook = hook
    mod.set_axon_ntff_profile_hook = lambda h: setattr(mod, "_hook", h)
    mod.get_axon_ntff_profile_hook = lambda: mod._hook
    _sys.modules["antenv.axon_hooks"] = mod

    # avoid S3 uploads from the trace path in this zero-egress container
    import concourse.bass_utils as bu

    bu.upload_artifacts = lambda tmpdir: f"local:{tmpdir}"


def run(inputs, trace=False, trace_cores=None):
    """Run on hardware; returns (full_output [B,T,V] f32, BassKernelResults)."""
    from concourse.bass_utils import run_bass_kernel_spmd

    if trace:
        _install_ntff_hook()

    nc = _get_program()
    in_maps = _make_in_maps(
        inputs["features"],
        inputs["captions"],
        inputs["embed_table"],
        inputs["W_ih"],
        inputs["W_hh"],
        inputs["b_ih"],
        inputs["b_hh"],
        inputs["fc_W"],
        inputs["fc_b"],
    )
    kwargs = {}
    if trace:
        import os
        import shutil

        shutil.rmtree("/tmp/bass_trace", ignore_errors=True)
        os.makedirs("/tmp/bass_trace", exist_ok=True)
        kwargs.update(trace=True, trace_cores=trace_cores or [0], tmpdir="/tmp/bass_trace")
    res = run_bass_kernel_spmd(nc, in_maps, core_ids=list(range(N_CORES)), **kwargs)
    full = np.concatenate(
        [np.asarray(r["out"], dtype=np.float32) for r in res.results], axis=2
    )
    return full, res


def kernel(**inputs) -> np.ndarray:
    out, _ = run(inputs, trace=False)
    return out
